# revision 15
# baseline (speedup 1.0000x reference)
"""Single-head causal attention kernel for Trainium2, 8-core data parallel.

Problem: x [8, 2048, 1024], Wk/Wq/Wv [64, 1024] ->
  out[b] = softmax(causal((x[b] @ Wq.T) @ (x[b] @ Wk.T).T / 8)) @ (x[b] @ Wv.T)

Sharding: one batch element per NeuronCore (data parallel across batch).

Per-core dataflow (bf16 PE operands, fp32 PSUM accumulation):
  - host supplies xT = x[b].T [1024, 2048] bf16; weights packed:
    wqk = [Wq.T | Wk.T] -> projection gives qT rows 0:64, kT rows 64:128.
  - x streams as [128, 1024] tiles for t [0,1024) (2 KB descriptor lines,
    better DMA rate) and [128, 512] tiles for subchunks 2, 3 (finer
    arrival staggering for the tail).
  - v is projected with COLUMN-TILED pairs (two concurrent matmuls on
    array col-halves, even e-tiles -> partitions 0:64, odd -> 64:128);
    halves are folded after the PE transpose by DVE adds.
  - a PE "swap" matmul (permutation stationary) produces [kT | qT] so
    scores get kT on partitions 0:64 without a DMA partition remap.
  - scores are computed TRANSPOSED, sT[t_k, t_q] = k_j.T @ q, so exp
    needs no max-subtraction and P feeds the output matmul as the
    moving operand:  out_psum[65, t_q] += ve_j.T @ P_j   where
    ve = [v | ones], the ones column making softmax row-sums a free
    65th output row.  The output matmuls run as THREE concurrent
    column-tiled matmuls (v[:,0:32] / v[:,32:64] / ones).
  - t_q is processed in 4 chunks of 512; chunk c uses key blocks
    j = 0..4c+3.  Scores for a PAIR of key blocks land side by side in
    one [128, 1024] PSUM tile and are exp'd with ONE ACTIVATE
    (halves the ACT per-call overhead).  Score matmuls are full-width
    (the below-diagonal columns compute garbage that the output
    matmuls simply never read); the diagonal 128-block is masked
    (0/1 upper-tri) after exp on DVE.
  - device output is unnormalized [65, 2048] fp32; host divides by the
    sums row and transposes.
"""
import sys

for _p in ("/opt/trn_rl_repo",):
    if _p not in sys.path:
        sys.path.insert(0, _p)

import numpy as np
import ml_dtypes
from contextlib import ExitStack

import concourse.bass as bass
import concourse.tile as tile
from concourse import bacc, mybir
from concourse.bass_utils import run_bass_kernel_spmd

FP = mybir.dt.float32
BF = mybir.dt.bfloat16
B, T, E, H = 8, 2048, 1024, 64
NE = E // 128            # 8 e-tiles (contraction)
SUB = 512                # projection subchunk = attention t_q chunk
NS = T // SUB            # 4
SCALE = 1.0 / np.sqrt(H)  # 0.125

_CACHE = {}


def _build_nc(do_compile=True):
    nc = bacc.Bacc(None, target_bir_lowering=False, debug=False)

    # xt is host-rearranged to [s, p, (e t)]: each 512-col subchunk is a
    # CONTIGUOUS 1.2 MB region -> one dma_start with 8 KB per-partition
    # lines (~340 GB/s vs ~180 for 1 KB lines), arriving staggered s0..s3.
    xt_d = nc.dram_tensor("xt", [NS * 128, NE * SUB], BF, kind="ExternalInput")
    wqk_d = nc.dram_tensor("wqk", [128, NE * 128], BF, kind="ExternalInput")
    wv_d = nc.dram_tensor("wv", [128, NE * H], BF, kind="ExternalInput")
    cst_d = nc.dram_tensor("cst", [128, 3 * 128], BF, kind="ExternalInput")
    out_d = nc.dram_tensor("out", [H + 1, T], FP, kind="ExternalOutput")

    with tile.TileContext(nc) as tc, ExitStack() as ctx:
        const = ctx.enter_context(tc.tile_pool(name="const", bufs=1))
        stripes = ctx.enter_context(tc.tile_pool(name="stripe", bufs=3))
        pp = ctx.enter_context(
            tc.tile_pool(name="pp", bufs=2, space=bass.MemorySpace.PSUM))
        sps = ctx.enter_context(
            tc.tile_pool(name="sps", bufs=2, space=bass.MemorySpace.PSUM))
        ops = ctx.enter_context(
            tc.tile_pool(name="ops", bufs=2, space=bass.MemorySpace.PSUM))

        # ---- SBUF tensors ----
        wqk_sb = const.tile([128, NE * 128], BF)
        wv_sb = const.tile([128, NE * H], BF)
        cst_sb = const.tile([128, 3 * 128], BF)
        perm = cst_sb[:, 0:128]
        ident = cst_sb[:, 128:256]
        mask = cst_sb[:, 256:384]
        junk = const.tile([128, SUB], BF)
        # one [128, 4096] tile per subchunk, layout [p, (e t)]
        xts = [const.tile([128, NE * SUB], BF, name=f"xts{s}") for s in range(NS)]
        qks = [const.tile([128, SUB], BF, name=f"qks{s}") for s in range(NS)]
        qsw = [const.tile([128, SUB], BF, name=f"qsw{s}") for s in range(NS)]
        vsb = [const.tile([128, SUB], BF, name=f"vsb{s}") for s in range(NS)]
        ve = [const.tile([128, H + 1], BF, name=f"ve{t}") for t in range(T // 128)]
        outc = [const.tile([H + 1, SUB], FP, name=f"outc{c}") for c in range(NS)]

        def xap(s, e):
            return xts[s][:, SUB * e:SUB * (e + 1)]

        # ---- DVE init (no DMA deps) + PE warmup junk matmuls ----
        nc.vector.memset(junk[:], 0.0)
        for t in range(T // 128):
            nc.vector.memset(ve[t][:, H:H + 1], 1.0)

        # ---- input DMAs: weights then x subchunks, all on the sync ring
        # (FIFO per ring -> back-to-back streaming, in-order completion).
        # Output DMAs use the scalar ring so they never queue behind x.
        nc.sync.dma_start(wqk_sb[:], wqk_d.ap())
        nc.sync.dma_start(wv_sb[:], wv_d.ap())
        nc.sync.dma_start(cst_sb[:], cst_d.ap())
        xt_ap = xt_d.ap()
        for s in range(NS):
            nc.sync.dma_start(xts[s][:], xt_ap[128 * s:128 * (s + 1), :])

        # PE warmup: lift the HAM clock gate while x streams in
        wu_ps = pp.tile([128, SUB], FP, tag="pp", name="wu_ps")
        for i in range(8):
            nc.tensor.matmul(wu_ps[:], junk[:, 0:128], junk[:],
                             start=True, stop=True, skip_group_check=True)

        qk_chain = {}

        def proj_qk_mm(s, e):
            """One e-tile of the qk projection chain for subchunk s."""
            if s not in qk_chain:
                qk_chain[s] = pp.tile([128, SUB], FP, tag="pp", name=f"qk_ps{s}")
            nc.tensor.matmul(
                qk_chain[s][:], wqk_sb[:, 128 * e:128 * (e + 1)], xap(s, e),
                start=(e == 0), stop=(e == NE - 1), skip_group_check=True)

        def proj_qk_fin(s):
            """Cast + swap for subchunk s (after its 8 chain matmuls)."""
            nc.vector.tensor_copy(qks[s][:], qk_chain[s][:])
            sw_ps = pp.tile([128, SUB], FP, tag="pp", name="sw_ps")
            nc.tensor.matmul(sw_ps[:], perm, qks[s][:], start=True, stop=True)
            nc.vector.tensor_copy(qsw[s][:], sw_ps[:])

        def proj_v(s):
            """Col-tiled paired v projection + transpose-fold for subchunk s."""
            v_ps = pp.tile([128, SUB], FP, tag="pp", name="v_ps")
            for ep in range(4):
                nc.tensor.matmul(
                    v_ps[0:64, :], wv_sb[:, H * 2 * ep:H * (2 * ep + 1)],
                    xap(s, 2 * ep),
                    start=(ep == 0), stop=(ep == 3), skip_group_check=True)
                nc.tensor.matmul(
                    v_ps[64:128, :], wv_sb[:, H * (2 * ep + 1):H * (2 * ep + 2)],
                    xap(s, 2 * ep + 1),
                    start=(ep == 0), stop=(ep == 3), skip_group_check=True,
                    tile_position=(0, 64))
            nc.vector.tensor_copy(vsb[s][:], v_ps[:])
            for ti in range(4):
                t = 4 * s + ti
                tr_ps = pp.tile([128, 128], BF, tag="pp", name="tr_ps")
                nc.tensor.transpose(
                    tr_ps[:], vsb[s][:, 128 * ti:128 * (ti + 1)], ident)
                # DVE may read only ONE operand from PSUM per instruction
                nc.vector.tensor_copy(ve[t][:, 0:H], tr_ps[:, 0:64])
                nc.vector.tensor_add(ve[t][:, 0:H], ve[t][:, 0:H],
                                     tr_ps[:, 64:128])

        def proj(s):
            for e in range(NE):
                proj_qk_mm(s, e)
            proj_qk_fin(s)
            proj_v(s)

        def attn_chunk(c, fillers=()):
            """Attention for t_q chunk c: key-block pieces j=0..4c+3 in pairs.

            fillers: optional callables run between pair-groups (to
            interleave next-subchunk projection work into PE's stream).
            """
            fi = 0
            out_ps = ops.tile([H + 1, SUB], FP, tag="ops", name=f"out_ps{c}")
            prev = None
            npiece = 4 * c + 4
            for g in range(npiece // 2):
                ja, jb = 2 * g, 2 * g + 1
                s_ps = sps.tile([128, 2 * SUB], FP, tag="sps", name="s_ps")
                # ROW-PAIRED scores: even piece on array rows 0:64 (kT from
                # the swap + qT original), odd piece on rows 64:128 (kT
                # original + qT from the swap).  Concurrent row-tiles double
                # throughput AND let each LDWEIGHTS overlap the other tile's
                # matmul (same-row-group LDW would serialize).
                # full-width scores: cols [0, n0) compute garbage that the
                # output matmuls never read.
                for half, j in ((0, ja), (1, jb)):
                    sj, bo = j // 4, 128 * (j % 4)
                    if half == 0:
                        nc.tensor.matmul(
                            s_ps[:, 0:SUB],
                            qsw[sj][0:64, bo:bo + 128], qks[c][0:64, :],
                            start=True, stop=True)
                    else:
                        nc.tensor.matmul(
                            s_ps[:, SUB:2 * SUB],
                            qks[sj][64:128, bo:bo + 128], qsw[c][64:128, :],
                            start=True, stop=True)
                if fi < len(fillers):
                    fillers[fi]()
                    fi += 1
                stripe = stripes.tile([128, 2 * SUB], BF, tag="stripe",
                                      name="stripe")
                nc.scalar.activation(
                    stripe[:], s_ps[:],
                    mybir.ActivationFunctionType.Exp, scale=float(SCALE))
                for half, j in ((0, ja), (1, jb)):
                    if j >= 4 * c:  # diagonal block in this chunk
                        n0 = 128 * j - SUB * c
                        nc.vector.tensor_mul(
                            stripe[:, SUB * half + n0:SUB * half + n0 + 128],
                            stripe[:, SUB * half + n0:SUB * half + n0 + 128],
                            mask)
                if prev is not None:
                    emit_outs(c, out_ps, *prev)
                prev = (ja, jb, stripe)
            emit_outs(c, out_ps, *prev)
            for f in fillers[fi:]:
                f()
            if c == NS - 1:
                # ACT is idle after the last exp; DVE may still be draining
                nc.scalar.copy(outc[c][:], out_ps[:])
            else:
                nc.vector.tensor_copy(outc[c][:], out_ps[:])
            nc.scalar.dma_start(out_d.ap()[:, SUB * c:SUB * (c + 1)], outc[c][:])

        def emit_outs(c, out_ps, ja, jb, stripe):
            for half, j in ((0, ja), (1, jb)):
                n0 = max(0, 128 * j - SUB * c)
                w0 = SUB * half + n0
                # three concurrent column-tiled matmuls: v lo / v hi / ones
                for (c0, c1) in ((0, 32), (32, 64), (64, 65)):
                    nc.tensor.matmul(
                        out_ps[c0:c1, n0:SUB], ve[j][:, c0:c1],
                        stripe[:, w0:SUB * (half + 1)],
                        start=(j == 0), stop=(j == 4 * c + 3),
                        skip_group_check=True)

        # ---- main schedule ----
        proj(0)
        proj(1)
        attn_chunk(0)
        attn_chunk(1, fillers=(lambda: proj(2),))

        # proj(3) interleaved into attn chunk 2 as fillers: the qk chain
        # rides the arriving x3 tiles, then cast+swap, then v.
        p3 = ([lambda e=e: proj_qk_mm(3, e) for e in range(NE)]
              + [lambda: proj_qk_fin(3), lambda: proj_v(3)])
        # 6 groups in chunk 2 -> bundle the 10 fillers into 6
        bundles = [p3[0:2], p3[2:4], p3[4:6], p3[6:8], p3[8:9], p3[9:10]]

        def mk(b):
            return lambda: [f() for f in b]
        attn_chunk(2, fillers=tuple(mk(b) for b in bundles))
        attn_chunk(3)

    if do_compile:
        nc.compile()
    return nc


def _get_nc():
    if "nc" not in _CACHE:
        _CACHE["nc"] = _build_nc()
    return _CACHE["nc"]


def _host_inputs(x, Wk, Wq, Wv):
    bf = ml_dtypes.bfloat16
    wqkT = np.concatenate([Wq.T, Wk.T], axis=1)            # [E, 128]
    wqk = np.ascontiguousarray(
        wqkT.reshape(NE, 128, 128).transpose(1, 0, 2).reshape(128, NE * 128)
    ).astype(bf)
    wvT = Wv.T                                             # [E, 64]
    wv = np.ascontiguousarray(
        wvT.reshape(NE, 128, H).transpose(1, 0, 2).reshape(128, NE * H)
    ).astype(bf)
    z = np.zeros((64, 64), np.float32)
    i64 = np.eye(64, dtype=np.float32)
    permh = np.block([[z, i64], [i64, z]])
    identh = np.eye(128, dtype=np.float32)
    maskh = np.triu(np.ones((128, 128), np.float32))       # keep t_k <= t_q
    cst = np.concatenate([permh, identh, maskh], axis=1).astype(bf)
    return wqk, wv, cst


def _xtr(xT):
    """xT [E, T] -> [NS*128, NE*SUB]: element (128e+p, 512s+t) lands at
    row 128s+p, col 512e+t — each subchunk contiguous, 8KB partition lines."""
    return np.ascontiguousarray(
        xT.reshape(NE, 128, NS, SUB).transpose(2, 1, 0, 3)
        .reshape(NS * 128, NE * SUB))


def _in_maps(x, Wk, Wq, Wv):
    wqk, wv, cst = _host_inputs(x, Wk, Wq, Wv)
    bf = ml_dtypes.bfloat16
    return [{
        "xt": _xtr(x[b].T.astype(bf)),
        "wqk": wqk, "wv": wv, "cst": cst,
    } for b in range(B)]


def kernel(x, Wk, Wq, Wv):
    x = np.ascontiguousarray(x, dtype=np.float32)
    assert x.shape == (B, T, E)
    nc = _get_nc()
    res = run_bass_kernel_spmd(nc, _in_maps(x, Wk, Wq, Wv), list(range(B)))
    out = np.empty((B, T, H), dtype=np.float32)
    for b in range(B):
        y = res.results[b]["out"]          # [65, T] unnormalized
        out[b] = (y[:H] / y[H:H + 1]).T
    return out


def run_traced(x, Wk, Wq, Wv):
    """Like kernel() but with NTFF profiling; returns (out, BassKernelResults)."""
    import types
    import antenv
    if "antenv.axon_hooks" not in sys.modules:
        hooks_mod = types.ModuleType("antenv.axon_hooks")
        _HOOK = [None]
        hooks_mod.set_axon_ntff_profile_hook = lambda h: _HOOK.__setitem__(0, h)
        hooks_mod.get_axon_ntff_profile_hook = lambda: _HOOK[0]
        sys.modules["antenv.axon_hooks"] = hooks_mod
        antenv.axon_hooks = hooks_mod
        from trn_agent_boot.trn_boot import _ntff_profile_via_ctypes
        hooks_mod.set_axon_ntff_profile_hook(
            _ntff_profile_via_ctypes("/opt/axon/libaxon_pjrt.so"))

    x = np.ascontiguousarray(x, dtype=np.float32)
    nc = _get_nc()
    res = run_bass_kernel_spmd(
        nc, _in_maps(x, Wk, Wq, Wv), list(range(B)), trace=True,
        trace_cores=[0])
    out = np.empty((B, T, H), dtype=np.float32)
    for b in range(B):
        y = res.results[b]["out"]
        out[b] = (y[:H] / y[H:H + 1]).T
    return out, res


# revision 16
# speedup vs baseline: 1.0583x; 1.0583x over previous
"""Single-head causal attention kernel for Trainium2, 8-core data parallel.

Problem: x [8, 2048, 1024], Wk/Wq/Wv [64, 1024] ->
  out[b] = softmax(causal((x[b] @ Wq.T) @ (x[b] @ Wk.T).T / 8)) @ (x[b] @ Wv.T)

Sharding: one batch element per NeuronCore (data parallel across batch).

Per-core dataflow (bf16 PE operands, fp32 PSUM accumulation):
  - host supplies xT = x[b].T [1024, 2048] bf16; weights packed:
    wqk = [Wq.T | Wk.T] -> projection gives qT rows 0:64, kT rows 64:128.
  - x streams as [128, 1024] tiles for t [0,1024) (2 KB descriptor lines,
    better DMA rate) and [128, 512] tiles for subchunks 2, 3 (finer
    arrival staggering for the tail).
  - v is projected with COLUMN-TILED pairs (two concurrent matmuls on
    array col-halves, even e-tiles -> partitions 0:64, odd -> 64:128);
    halves are folded after the PE transpose by DVE adds.
  - a PE "swap" matmul (permutation stationary) produces [kT | qT] so
    scores get kT on partitions 0:64 without a DMA partition remap.
  - scores are computed TRANSPOSED, sT[t_k, t_q] = k_j.T @ q, so exp
    needs no max-subtraction and P feeds the output matmul as the
    moving operand:  out_psum[65, t_q] += ve_j.T @ P_j   where
    ve = [v | ones], the ones column making softmax row-sums a free
    65th output row.  The output matmuls run as THREE concurrent
    column-tiled matmuls (v[:,0:32] / v[:,32:64] / ones).
  - t_q is processed in 4 chunks of 512; chunk c uses key blocks
    j = 0..4c+3.  Scores for a PAIR of key blocks land side by side in
    one [128, 1024] PSUM tile and are exp'd with ONE ACTIVATE
    (halves the ACT per-call overhead).  Score matmuls are full-width
    (the below-diagonal columns compute garbage that the output
    matmuls simply never read); the diagonal 128-block is masked
    (0/1 upper-tri) after exp on DVE.
  - device output is unnormalized [65, 2048] fp32; host divides by the
    sums row and transposes.
"""
import sys

for _p in ("/opt/trn_rl_repo",):
    if _p not in sys.path:
        sys.path.insert(0, _p)

import numpy as np
import ml_dtypes
from contextlib import ExitStack

import concourse.bass as bass
import concourse.tile as tile
from concourse import bacc, mybir
from concourse.bass_utils import run_bass_kernel_spmd

FP = mybir.dt.float32
BF = mybir.dt.bfloat16
B, T, E, H = 8, 2048, 1024, 64
NE = E // 128            # 8 e-tiles (contraction)
SUB = 512                # projection subchunk = attention t_q chunk
NS = T // SUB            # 4
SCALE = 1.0 / np.sqrt(H)  # 0.125

_CACHE = {}


def _build_nc(do_compile=True):
    nc = bacc.Bacc(None, target_bir_lowering=False, debug=False)

    # xt is host-rearranged to [s, p, (e t)]: each 512-col subchunk is a
    # CONTIGUOUS 1.2 MB region -> one dma_start with 8 KB per-partition
    # lines (~340 GB/s vs ~180 for 1 KB lines), arriving staggered s0..s3.
    xt_d = nc.dram_tensor("xt", [NS * 128, NE * SUB], BF, kind="ExternalInput")
    wqk_d = nc.dram_tensor("wqk", [128, NE * 128], BF, kind="ExternalInput")
    wv_d = nc.dram_tensor("wv", [128, NE * H], BF, kind="ExternalInput")
    cst_d = nc.dram_tensor("cst", [128, 3 * 128], BF, kind="ExternalInput")
    out_d = nc.dram_tensor("out", [H + 1, T], FP, kind="ExternalOutput")

    with tile.TileContext(nc) as tc, ExitStack() as ctx:
        const = ctx.enter_context(tc.tile_pool(name="const", bufs=1))
        stripes = ctx.enter_context(tc.tile_pool(name="stripe", bufs=3))
        pp = ctx.enter_context(
            tc.tile_pool(name="pp", bufs=2, space=bass.MemorySpace.PSUM))
        sps = ctx.enter_context(
            tc.tile_pool(name="sps", bufs=2, space=bass.MemorySpace.PSUM))
        ops = ctx.enter_context(
            tc.tile_pool(name="ops", bufs=2, space=bass.MemorySpace.PSUM))

        # ---- SBUF tensors ----
        wqk_sb = const.tile([128, NE * 128], BF)
        wv_sb = const.tile([128, NE * H], BF)
        cst_sb = const.tile([128, 3 * 128], BF)
        perm = cst_sb[:, 0:128]
        ident = cst_sb[:, 128:256]
        mask = cst_sb[:, 256:384]
        junk = const.tile([128, SUB], BF)
        # one [128, 4096] tile per subchunk, layout [p, (e t)]
        xts = [const.tile([128, NE * SUB], BF, name=f"xts{s}") for s in range(NS)]
        qks = [const.tile([128, SUB], BF, name=f"qks{s}") for s in range(NS)]
        qsw = [const.tile([128, SUB], BF, name=f"qsw{s}") for s in range(NS)]
        vsb = [const.tile([128, SUB], BF, name=f"vsb{s}") for s in range(NS)]
        ve = [const.tile([128, H + 1], BF, name=f"ve{t}") for t in range(T // 128)]
        outc = [const.tile([H + 1, SUB], FP, name=f"outc{c}") for c in range(NS)]

        def xap(s, e):
            return xts[s][:, SUB * e:SUB * (e + 1)]

        # ---- DVE init (no DMA deps) + PE warmup junk matmuls ----
        nc.vector.memset(junk[:], 0.0)
        for t in range(T // 128):
            nc.vector.memset(ve[t][:, H:H + 1], 1.0)

        # ---- input DMAs: weights then x subchunks, all on the sync ring
        # (FIFO per ring -> back-to-back streaming, in-order completion).
        # Output DMAs use the scalar ring so they never queue behind x.
        # x0 issued FIRST (each dma_start costs ~625ns of serial sequencer
        # issue; x0 gates everything), weights interleaved behind it.
        xt_ap = xt_d.ap()
        nc.sync.dma_start(xts[0][:], xt_ap[0:128, :])
        nc.sync.dma_start(wqk_sb[:], wqk_d.ap())
        nc.sync.dma_start(xts[1][:], xt_ap[128:256, :])
        nc.sync.dma_start(wv_sb[:], wv_d.ap())
        nc.sync.dma_start(cst_sb[:], cst_d.ap())
        for s in range(2, NS):
            nc.sync.dma_start(xts[s][:], xt_ap[128 * s:128 * (s + 1), :])

        # PE warmup: lift the HAM clock gate while x streams in (N=256
        # pieces so the tail of the junk run delays proj0 minimally)
        wu_ps = pp.tile([128, SUB], FP, tag="pp", name="wu_ps")
        for i in range(12):
            nc.tensor.matmul(wu_ps[:, 0:256], junk[:, 0:128], junk[:, 0:256],
                             start=True, stop=True, skip_group_check=True)

        qk_chain = {}

        def proj_qk_mm(s, e):
            """One e-tile of the qk projection chain for subchunk s."""
            if s not in qk_chain:
                qk_chain[s] = pp.tile([128, SUB], FP, tag="pp", name=f"qk_ps{s}")
            nc.tensor.matmul(
                qk_chain[s][:], wqk_sb[:, 128 * e:128 * (e + 1)], xap(s, e),
                start=(e == 0), stop=(e == NE - 1), skip_group_check=True)

        def proj_qk_fin(s):
            """Cast + swap for subchunk s (after its 8 chain matmuls)."""
            nc.vector.tensor_copy(qks[s][:], qk_chain[s][:])
            sw_ps = pp.tile([128, SUB], FP, tag="pp", name="sw_ps")
            nc.tensor.matmul(sw_ps[:], perm, qks[s][:], start=True, stop=True)
            nc.vector.tensor_copy(qsw[s][:], sw_ps[:])

        def proj_v(s):
            """Col-tiled paired v projection + transpose-fold for subchunk s."""
            v_ps = pp.tile([128, SUB], FP, tag="pp", name="v_ps")
            for ep in range(4):
                nc.tensor.matmul(
                    v_ps[0:64, :], wv_sb[:, H * 2 * ep:H * (2 * ep + 1)],
                    xap(s, 2 * ep),
                    start=(ep == 0), stop=(ep == 3), skip_group_check=True)
                nc.tensor.matmul(
                    v_ps[64:128, :], wv_sb[:, H * (2 * ep + 1):H * (2 * ep + 2)],
                    xap(s, 2 * ep + 1),
                    start=(ep == 0), stop=(ep == 3), skip_group_check=True,
                    tile_position=(0, 64))
            nc.vector.tensor_copy(vsb[s][:], v_ps[:])
            for ti in range(4):
                t = 4 * s + ti
                tr_ps = pp.tile([128, 128], BF, tag="pp", name="tr_ps")
                nc.tensor.transpose(
                    tr_ps[:], vsb[s][:, 128 * ti:128 * (ti + 1)], ident)
                # DVE may read only ONE operand from PSUM per instruction
                nc.vector.tensor_copy(ve[t][:, 0:H], tr_ps[:, 0:64])
                nc.vector.tensor_add(ve[t][:, 0:H], ve[t][:, 0:H],
                                     tr_ps[:, 64:128])

        def proj(s):
            for e in range(NE):
                proj_qk_mm(s, e)
            proj_qk_fin(s)
            proj_v(s)

        def attn_chunk(c, fillers=()):
            """Attention for t_q chunk c: key-block pieces j=0..4c+3 in pairs.

            fillers: optional callables run between pair-groups (to
            interleave next-subchunk projection work into PE's stream).
            """
            fi = 0
            out_ps = ops.tile([H + 1, SUB], FP, tag="ops", name=f"out_ps{c}")
            prev = None
            npiece = 4 * c + 4
            for g in range(npiece // 2):
                ja, jb = 2 * g, 2 * g + 1
                s_ps = sps.tile([128, 2 * SUB], FP, tag="sps", name="s_ps")
                # ROW-PAIRED scores: even piece on array rows 0:64 (kT from
                # the swap + qT original), odd piece on rows 64:128 (kT
                # original + qT from the swap).  Concurrent row-tiles double
                # throughput AND let each LDWEIGHTS overlap the other tile's
                # matmul (same-row-group LDW would serialize).
                # full-width scores: cols [0, n0) compute garbage that the
                # output matmuls never read.
                for half, j in ((0, ja), (1, jb)):
                    sj, bo = j // 4, 128 * (j % 4)
                    if half == 0:
                        nc.tensor.matmul(
                            s_ps[:, 0:SUB],
                            qsw[sj][0:64, bo:bo + 128], qks[c][0:64, :],
                            start=True, stop=True)
                    else:
                        nc.tensor.matmul(
                            s_ps[:, SUB:2 * SUB],
                            qks[sj][64:128, bo:bo + 128], qsw[c][64:128, :],
                            start=True, stop=True)
                if fi < len(fillers):
                    fillers[fi]()
                    fi += 1
                stripe = stripes.tile([128, 2 * SUB], BF, tag="stripe",
                                      name="stripe")
                nc.scalar.activation(
                    stripe[:], s_ps[:],
                    mybir.ActivationFunctionType.Exp, scale=float(SCALE))
                for half, j in ((0, ja), (1, jb)):
                    if j >= 4 * c:  # diagonal block in this chunk
                        n0 = 128 * j - SUB * c
                        nc.vector.tensor_mul(
                            stripe[:, SUB * half + n0:SUB * half + n0 + 128],
                            stripe[:, SUB * half + n0:SUB * half + n0 + 128],
                            mask)
                if prev is not None:
                    emit_outs(c, out_ps, *prev)
                prev = (ja, jb, stripe)
            emit_outs(c, out_ps, *prev)
            for f in fillers[fi:]:
                f()
            if c == NS - 1:
                # ACT is idle after the last exp; DVE may still be draining
                nc.scalar.copy(outc[c][:], out_ps[:])
            else:
                nc.vector.tensor_copy(outc[c][:], out_ps[:])
            nc.scalar.dma_start(out_d.ap()[:, SUB * c:SUB * (c + 1)], outc[c][:])

        def emit_outs(c, out_ps, ja, jb, stripe):
            for half, j in ((0, ja), (1, jb)):
                n0 = max(0, 128 * j - SUB * c)
                w0 = SUB * half + n0
                # three concurrent column-tiled matmuls: v lo / v hi / ones
                for (c0, c1) in ((0, 32), (32, 64), (64, 65)):
                    nc.tensor.matmul(
                        out_ps[c0:c1, n0:SUB], ve[j][:, c0:c1],
                        stripe[:, w0:SUB * (half + 1)],
                        start=(j == 0), stop=(j == 4 * c + 3),
                        skip_group_check=True)

        # ---- main schedule ----
        proj(0)
        proj(1)
        attn_chunk(0)
        attn_chunk(1, fillers=(lambda: proj(2),))

        # proj(3) interleaved into attn chunk 2 as fillers: the qk chain
        # rides the arriving x3 tiles, then cast+swap, then v.
        p3 = ([lambda e=e: proj_qk_mm(3, e) for e in range(NE)]
              + [lambda: proj_qk_fin(3), lambda: proj_v(3)])
        # 6 groups in chunk 2 -> bundle the 10 fillers into 6
        bundles = [p3[0:2], p3[2:4], p3[4:6], p3[6:8], p3[8:9], p3[9:10]]

        def mk(b):
            return lambda: [f() for f in b]
        attn_chunk(2, fillers=tuple(mk(b) for b in bundles))
        attn_chunk(3)

    if do_compile:
        nc.compile()
    return nc


def _get_nc():
    if "nc" not in _CACHE:
        _CACHE["nc"] = _build_nc()
    return _CACHE["nc"]


def _host_inputs(x, Wk, Wq, Wv):
    bf = ml_dtypes.bfloat16
    wqkT = np.concatenate([Wq.T, Wk.T], axis=1)            # [E, 128]
    wqk = np.ascontiguousarray(
        wqkT.reshape(NE, 128, 128).transpose(1, 0, 2).reshape(128, NE * 128)
    ).astype(bf)
    wvT = Wv.T                                             # [E, 64]
    wv = np.ascontiguousarray(
        wvT.reshape(NE, 128, H).transpose(1, 0, 2).reshape(128, NE * H)
    ).astype(bf)
    z = np.zeros((64, 64), np.float32)
    i64 = np.eye(64, dtype=np.float32)
    permh = np.block([[z, i64], [i64, z]])
    identh = np.eye(128, dtype=np.float32)
    maskh = np.triu(np.ones((128, 128), np.float32))       # keep t_k <= t_q
    cst = np.concatenate([permh, identh, maskh], axis=1).astype(bf)
    return wqk, wv, cst


def _xtr(xT):
    """xT [E, T] -> [NS*128, NE*SUB]: element (128e+p, 512s+t) lands at
    row 128s+p, col 512e+t — each subchunk contiguous, 8KB partition lines."""
    return np.ascontiguousarray(
        xT.reshape(NE, 128, NS, SUB).transpose(2, 1, 0, 3)
        .reshape(NS * 128, NE * SUB))


def _in_maps(x, Wk, Wq, Wv):
    wqk, wv, cst = _host_inputs(x, Wk, Wq, Wv)
    bf = ml_dtypes.bfloat16
    return [{
        "xt": _xtr(x[b].T.astype(bf)),
        "wqk": wqk, "wv": wv, "cst": cst,
    } for b in range(B)]


def kernel(x, Wk, Wq, Wv):
    x = np.ascontiguousarray(x, dtype=np.float32)
    assert x.shape == (B, T, E)
    nc = _get_nc()
    res = run_bass_kernel_spmd(nc, _in_maps(x, Wk, Wq, Wv), list(range(B)))
    out = np.empty((B, T, H), dtype=np.float32)
    for b in range(B):
        y = res.results[b]["out"]          # [65, T] unnormalized
        out[b] = (y[:H] / y[H:H + 1]).T
    return out


def run_traced(x, Wk, Wq, Wv):
    """Like kernel() but with NTFF profiling; returns (out, BassKernelResults)."""
    import types
    import antenv
    if "antenv.axon_hooks" not in sys.modules:
        hooks_mod = types.ModuleType("antenv.axon_hooks")
        _HOOK = [None]
        hooks_mod.set_axon_ntff_profile_hook = lambda h: _HOOK.__setitem__(0, h)
        hooks_mod.get_axon_ntff_profile_hook = lambda: _HOOK[0]
        sys.modules["antenv.axon_hooks"] = hooks_mod
        antenv.axon_hooks = hooks_mod
        from trn_agent_boot.trn_boot import _ntff_profile_via_ctypes
        hooks_mod.set_axon_ntff_profile_hook(
            _ntff_profile_via_ctypes("/opt/axon/libaxon_pjrt.so"))

    x = np.ascontiguousarray(x, dtype=np.float32)
    nc = _get_nc()
    res = run_bass_kernel_spmd(
        nc, _in_maps(x, Wk, Wq, Wv), list(range(B)), trace=True,
        trace_cores=[0])
    out = np.empty((B, T, H), dtype=np.float32)
    for b in range(B):
        y = res.results[b]["out"]
        out[b] = (y[:H] / y[H:H + 1]).T
    return out, res
